# revision 10
# baseline (speedup 1.0000x reference)
"""Trainium2 Bass kernel for the custom RNN, v2.

    A_t = tanh(x_t @ W1 + b1)              # batched over (T, B)  - phase A
    y_t = A_t + tanh(y_{t-1} @ W2 + b2)    # sequential           - chains
    out_t = y_t.Wc                         # batched per block    - phase C

Sharding: data-parallel over batch B=128 -> 16 per core across 8 cores.

The sequential phase runs as G=2 staggered chains (batch 8 each) so the
Activation engine's fixed per-instruction latency on one chain overlaps
the other chain's matmuls. Per chain step: one 4-row matmul preloads the
PSUM with b2 (b2rows x indicator), 16 W2 matmuls accumulate y_{t-1}@W2,
one tanh (ACT) and one residual add vs A_t (DVE) produce y_t. Weight
loads dominate the PE on hardware, so no batched C=A@W2 precompute.

Phase A/C work for neighboring blocks is paced into the chain loop a few
instructions per step - engines execute their queues in program order, so
emitting a block's worth of batched matmuls contiguously would stall the
chains.
"""

import os
from collections import deque

import numpy as np

import concourse.bass as bass
import concourse.bacc as bacc
import concourse.mybir as mybir
from concourse.tile import TileContext
from concourse.bass_utils import run_bass_kernel_spmd

B, T_FULL, D, U = 128, 2048, 512, 512
NCORES = 8
BS = B // NCORES          # 16 batch rows per core
G = 2                     # staggered chains per core
BG = BS // G              # batch rows per chain
TBLK = 32                 # time steps per block
KB = D // 128             # contraction chunks
MB = U // 128             # output-feature chunks
BF = mybir.dt.bfloat16
F32 = mybir.dt.float32
TANH = mybir.ActivationFunctionType.Tanh


def build_nc(t_total=T_FULL):
    assert t_total % TBLK == 0
    nblk = t_total // TBLK
    nc = bacc.Bacc(target_bir_lowering=False)

    x = nc.dram_tensor("x", [BS, t_total, D], F32, kind="ExternalInput")
    W1 = nc.dram_tensor("W1", [D, U], F32, kind="ExternalInput")
    b1 = nc.dram_tensor("b1", [U], F32, kind="ExternalInput")
    W2 = nc.dram_tensor("W2", [U, U], F32, kind="ExternalInput")
    b2 = nc.dram_tensor("b2", [U], F32, kind="ExternalInput")
    Wc = nc.dram_tensor("Wc", [U, 1], F32, kind="ExternalInput")
    # flat time-major output: free index = t*BS + beta; host transposes
    out = nc.dram_tensor("out", [1, t_total * BS], F32, kind="ExternalOutput")

    with TileContext(nc) as tc:
        with (
            tc.tile_pool(name="const", bufs=1) as cpool,
            tc.tile_pool(name="x32", bufs=2) as x32pool,
            tc.tile_pool(name="xbm", bufs=2) as xpool,
            tc.tile_pool(name="xt", bufs=2) as xtpool,
            tc.tile_pool(name="at", bufs=3) as apool,
            tc.tile_pool(name="ut", bufs=3) as upool,
            tc.tile_pool(name="tt", bufs=3) as tpool,
            tc.tile_pool(name="psa", bufs=2, space="PSUM") as pa_pool,
            tc.tile_pool(name="pso", bufs=2, space="PSUM") as po_pool,
            tc.tile_pool(name="psz", bufs=1, space="PSUM") as pz_pool,
        ):
            # ---- persistent weights / biases (cast to bf16 during DMA) ----
            w1sb = cpool.tile([128, KB * MB * 128], BF)   # (k, m, mi)
            w2sb = cpool.tile([128, KB * MB * 128], BF)
            b1sb = cpool.tile([128, MB], F32)
            b2sb = cpool.tile([128, MB], F32)
            wcsb = cpool.tile([128, KB], BF)

            w1v = w1sb.rearrange("p (kb mb mi) -> p kb mb mi", kb=KB, mb=MB)
            w2v = w2sb.rearrange("p (kb mb mi) -> p kb mb mi", kb=KB, mb=MB)

            nc.gpsimd.dma_start(
                out=w1v, in_=W1.rearrange("(kb p) (mb mi) -> p kb mb mi", p=128, mi=128)
            )
            nc.gpsimd.dma_start(
                out=w2v, in_=W2.rearrange("(kb p) (mb mi) -> p kb mb mi", p=128, mi=128)
            )
            nc.sync.dma_start(out=b1sb, in_=b1.rearrange("(mb p) -> p mb", p=128))
            nc.sync.dma_start(out=b2sb, in_=b2.rearrange("(mb p) -> p mb", p=128))
            nc.gpsimd.dma_start(
                out=wcsb, in_=Wc.rearrange("(kb p) one -> p (kb one)", p=128)
            )

            # b2 as 4 contraction rows + per-m indicator: one matmul
            # b2rows.T @ ind = b2[m*128+p] broadcast over batch, preloading
            # the chain PSUM with the bias at a 4-row LdWeights cost
            b2rows = cpool.tile([4, 128], BF)
            nc.gpsimd.dma_start(
                out=b2rows, in_=b2.rearrange("(j c) -> j c", j=4)
            )
            ind4 = cpool.tile([4, MB * BG], BF)
            nc.vector.memset(ind4, 1.0)
            nc.gpsimd.affine_select(
                ind4, ind4, pattern=[[-1, MB], [0, BG]],
                compare_op=mybir.AluOpType.is_equal, fill=0.0,
                base=0, channel_multiplier=1,
            )

            # initial state: y_0 = 0
            y0 = cpool.tile([128, MB * BS], BF)
            nc.vector.memset(y0, 0.0)
            y0v = y0.rearrange("p (mb beta) -> p mb beta", mb=MB)

            # staging for the whole flat output (lives on one partition)
            osb = cpool.tile([1, t_total * BS], F32)

            # chain PSUM accumulators, one per chain (persistent)
            zps = [
                pz_pool.tile([128, MB * BG], F32, name=f"zps{g}") for g in range(G)
            ]
            zpv = [
                z.rearrange("p (mb beta) -> p mb beta", mb=MB) for z in zps
            ]

            av, cv, uv = {}, {}, {}

            def phase_a_ops(b):
                """Closures computing A(b) and C(b); FIFO order = dep order."""
                t0 = b * TBLK
                ops = []
                # fp32 load on the HW DGE (the software DGE that can cast is
                # ~10x slower); GPSIMD then casts to bf16 for the transposes
                xbm32 = x32pool.tile([128, 4 * D], F32, name="xbm32")
                xbm32_v = xbm32.rearrange("p (i d) -> p i d", i=4)
                xbm = xpool.tile([128, 4 * D], BF, name="xbm")
                xbm_v = xbm.rearrange("p (i d) -> p i d", i=4)
                for i in range(4):
                    ops.append(lambda i=i: nc.sync.dma_start(
                        out=xbm32_v[:, i, :],
                        in_=x[:, t0 + 8 * i : t0 + 8 * (i + 1), :].rearrange(
                            "b tp d -> tp b d"
                        ),
                    ))
                    ops.append(lambda i=i: nc.gpsimd.tensor_copy(
                        xbm_v[:, i, :], xbm32_v[:, i, :]
                    ))
                xt = xtpool.tile([128, KB * 512], BF, name="xt")
                xt_v = xt.rearrange("p (kb bt) -> p kb bt", kb=KB)
                for i in range(4):
                    for dj in range(KB):
                        ops.append(lambda i=i, dj=dj: nc.sync.dma_start(
                            out=xt_v[:, dj, i * 128 : (i + 1) * 128],
                            in_=xbm_v[:, i, dj * 128 : (dj + 1) * 128],
                            transpose=True,
                        ))

                asb = apool.tile([128, TBLK * MB * BS], BF, name="asb")
                av[b] = asb.rearrange(
                    "p (tau mb beta) -> p tau mb beta", tau=TBLK, mb=MB
                )

                psa_t = {}

                def w1_mm(m, k):
                    if k == 0:
                        psa_t[m] = pa_pool.tile([128, 512], F32, name="psa")
                    nc.tensor.matmul(
                        psa_t[m], w1v[:, k, m, :], xt_v[:, k, :],
                        start=(k == 0), stop=(k == KB - 1),
                    )

                for m in range(MB):
                    for k in range(KB):
                        ops.append(lambda m=m, k=k: w1_mm(m, k))
                    ops.append(lambda m=m: nc.scalar.activation(
                        av[b][:, :, m, :],
                        psa_t[m].rearrange("p (tau beta) -> p tau beta", tau=TBLK),
                        TANH, bias=b1sb[:, m : m + 1],
                    ))

                return ops

            def phase_c_ops(b):
                """out_t = A_t.Wc + u_t.Wc for block b."""
                t0 = b * TBLK
                ops = []
                po_t = {}

                def o_mm(k):
                    if k == 0:
                        po_t[0] = po_pool.tile([1, TBLK * BS], F32, name="po")
                    nc.tensor.matmul(
                        po_t[0], wcsb[:, k : k + 1], uv[b][:, :, k, :],
                        start=(k == 0), stop=(k == KB - 1),
                    )

                for k in range(KB):
                    ops.append(lambda k=k: o_mm(k))
                ops.append(lambda: nc.vector.tensor_copy(
                    osb[:, t0 * BS : (t0 + TBLK) * BS], po_t[0]
                ))
                return ops

            def chain_pre(b, tau, g):
                """Open zps[g]'s accumulation group with the b2 preload."""
                nc.tensor.matmul(
                    zpv[g], b2rows, ind4,
                    start=True, stop=False,
                )

            def chain_main(b, tau, g):
                gs = slice(g * BG, (g + 1) * BG)
                if b == 0 and tau == 0:
                    y_src = lambda k: y0v[:, k, gs]
                elif tau == 0:
                    y_src = lambda k: uv[b - 1][:, TBLK - 1, k, gs]
                else:
                    y_src = lambda k: uv[b][:, tau - 1, k, gs]
                for k in range(KB):
                    for m in range(MB):
                        nc.tensor.matmul(
                            zpv[g][:, m, :], w2v[:, k, m, :], y_src(k),
                            start=False, stop=(k == KB - 1 and m == MB - 1),
                        )
                ttmp = tpool.tile([128, MB * BG], BF, name=f"ttmp{g}")
                nc.scalar.activation(ttmp, zpv[g], TANH)
                nc.vector.tensor_add(
                    uv[b][:, tau, :, gs],
                    ttmp.rearrange("p (mb beta) -> p mb beta", mb=MB),
                    av[b][:, tau, :, gs],
                )

            # block 0's phase A must fully precede its chains
            for op in phase_a_ops(0):
                op()

            pending = deque()
            for b in range(nblk):
                ublk = upool.tile([128, TBLK * MB * BS], BF, name="ublk")
                uv[b] = ublk.rearrange(
                    "p (tau mb beta) -> p tau mb beta", tau=TBLK, mb=MB
                )
                if b + 1 < nblk:
                    pending.extend(phase_a_ops(b + 1))
                if b >= 1:
                    pending.extend(phase_c_ops(b - 1))
                for tau in range(TBLK):
                    for g in range(G):
                        chain_pre(b, tau, g)
                        chain_main(b, tau, g)
                    take = -(-len(pending) // (TBLK - tau))
                    for _ in range(take):
                        pending.popleft()()
                # free stale views (pools recycle the buffers)
                for d in (av, cv, uv):
                    d.pop(b - 2, None)

            for op in phase_c_ops(nblk - 1):
                op()

            nc.sync.dma_start(out=out[:, :], in_=osb)

    nc.finalize()
    return nc


_CACHE = {}


def _get_nc(t_total):
    if t_total not in _CACHE:
        _CACHE[t_total] = build_nc(t_total)
    return _CACHE[t_total]


def _run(inputs, t_total, trace=False):
    nc = _get_nc(t_total)
    xf = np.ascontiguousarray(np.asarray(inputs["x"], dtype=np.float32))
    in_common = {
        k: np.ascontiguousarray(np.asarray(inputs[k], dtype=np.float32))
        for k in ("W1", "b1", "W2", "b2", "Wc")
    }
    in_maps = [
        {"x": xf[c * BS : (c + 1) * BS, :t_total], **in_common} for c in range(NCORES)
    ]
    res = run_bass_kernel_spmd(nc, in_maps, core_ids=list(range(NCORES)), trace=trace)
    outs = np.concatenate(
        [
            res.results[c]["out"].reshape(t_total, BS).T
            for c in range(NCORES)
        ],
        axis=0,
    )
    bc = np.asarray(inputs["bc"], dtype=np.float32)
    full = outs[:, :, None] + bc[None, None, :]
    return full.astype(np.float32), res


def kernel(**inputs):
    t_total = int(os.environ.get("RNN_T", str(T_FULL)))
    full, _ = _run(inputs, t_total, trace=False)
    return full


# revision 11
# speedup vs baseline: 1.1957x; 1.1957x over previous
"""Trainium2 Bass kernel for the custom RNN, v2.

    A_t = tanh(x_t @ W1 + b1)              # batched over (T, B)  - phase A
    C_t = A_t @ W2 + b2                    # batched over (T, B)  - phase A
    u_t = tanh(C_{t-1} + u_{t-1} @ W2)     # sequential           - chains
    y_t = A_t + u_t                        # never materialized
    out_t = A_t.Wc + u_t.Wc                # batched per block    - phase C

Sharding: data-parallel over batch B=128 -> 16 per core across 8 cores.

The sequential phase runs as G=2 staggered chains (batch 8 each) so the
Activation engine's fixed per-instruction latency on one chain overlaps
the other chain's matmuls. Per chain step: 4 identity-matmuls preload the
PSUM with C_{t-1} (b2 already folded in), 16 W2 matmuls accumulate
u_{t-1}@W2, and a single tanh activation writes u_t back to SBUF.

Phase A/C work for neighboring blocks is paced into the chain loop a few
instructions per step - engines execute their queues in program order, so
emitting a block's worth of batched matmuls contiguously would stall the
chains.
"""

import os
from collections import deque

import numpy as np

import concourse.bass as bass
import concourse.bacc as bacc
import concourse.mybir as mybir
from concourse.tile import TileContext
from concourse.bass_utils import run_bass_kernel_spmd

B, T_FULL, D, U = 128, 2048, 512, 512
NCORES = 8
BS = B // NCORES          # 16 batch rows per core
G = 2                     # staggered chains per core
BG = BS // G              # batch rows per chain
TBLK = 32                 # time steps per block
KB = D // 128             # contraction chunks
MB = U // 128             # output-feature chunks
BF = mybir.dt.bfloat16
F32 = mybir.dt.float32
TANH = mybir.ActivationFunctionType.Tanh


def build_nc(t_total=T_FULL):
    assert t_total % TBLK == 0
    nblk = t_total // TBLK
    nc = bacc.Bacc(target_bir_lowering=False)

    x = nc.dram_tensor("x", [BS, t_total, D], F32, kind="ExternalInput")
    W1 = nc.dram_tensor("W1", [D, U], F32, kind="ExternalInput")
    b1 = nc.dram_tensor("b1", [U], F32, kind="ExternalInput")
    W2 = nc.dram_tensor("W2", [U, U], F32, kind="ExternalInput")
    b2 = nc.dram_tensor("b2", [U], F32, kind="ExternalInput")
    Wc = nc.dram_tensor("Wc", [U, 1], F32, kind="ExternalInput")
    # flat time-major output: free index = t*BS + beta; host transposes
    out = nc.dram_tensor("out", [1, t_total * BS], F32, kind="ExternalOutput")

    with TileContext(nc) as tc:
        with (
            tc.tile_pool(name="const", bufs=1) as cpool,
            tc.tile_pool(name="x32", bufs=2) as x32pool,
            tc.tile_pool(name="xbm", bufs=2) as xpool,
            tc.tile_pool(name="xt", bufs=2) as xtpool,
            tc.tile_pool(name="at", bufs=3) as apool,
            tc.tile_pool(name="ct", bufs=3) as cblkpool,
            tc.tile_pool(name="ut", bufs=3) as upool,
            tc.tile_pool(name="psa", bufs=2, space="PSUM") as pa_pool,
            tc.tile_pool(name="psc", bufs=2, space="PSUM") as pc_pool,
            tc.tile_pool(name="pso", bufs=2, space="PSUM") as po_pool,
            tc.tile_pool(name="psz", bufs=1, space="PSUM") as pz_pool,
        ):
            # ---- persistent weights / biases (cast to bf16 during DMA) ----
            w1sb = cpool.tile([128, KB * MB * 128], BF)   # (k, m, mi)
            w2sb = cpool.tile([128, KB * MB * 128], BF)
            b1sb = cpool.tile([128, MB], F32)
            b2sb = cpool.tile([128, MB], F32)
            wcsb = cpool.tile([128, KB], BF)

            w1v = w1sb.rearrange("p (kb mb mi) -> p kb mb mi", kb=KB, mb=MB)
            w2v = w2sb.rearrange("p (kb mb mi) -> p kb mb mi", kb=KB, mb=MB)

            nc.gpsimd.dma_start(
                out=w1v, in_=W1.rearrange("(kb p) (mb mi) -> p kb mb mi", p=128, mi=128)
            )
            nc.gpsimd.dma_start(
                out=w2v, in_=W2.rearrange("(kb p) (mb mi) -> p kb mb mi", p=128, mi=128)
            )
            nc.sync.dma_start(out=b1sb, in_=b1.rearrange("(mb p) -> p mb", p=128))
            nc.sync.dma_start(out=b2sb, in_=b2.rearrange("(mb p) -> p mb", p=128))
            nc.gpsimd.dma_start(
                out=wcsb, in_=Wc.rearrange("(kb p) one -> p (kb one)", p=128)
            )

            # identity (for PSUM preload matmuls)
            ident = cpool.tile([128, 128], BF)
            nc.vector.memset(ident, 1.0)
            nc.gpsimd.affine_select(
                ident, ident, pattern=[[-1, 128]],
                compare_op=mybir.AluOpType.is_equal, fill=0.0,
                base=0, channel_multiplier=1,
            )

            # initial state: u_0 = 0, C_0 = b2 broadcast over batch
            u0 = cpool.tile([128, MB * BS], BF)
            nc.vector.memset(u0, 0.0)
            c0 = cpool.tile([128, MB * BS], BF)
            u0v = u0.rearrange("p (mb beta) -> p mb beta", mb=MB)
            c0v = c0.rearrange("p (mb beta) -> p mb beta", mb=MB)
            for m in range(MB):
                nc.vector.tensor_scalar_add(
                    c0v[:, m, :], u0v[:, m, :], b2sb[:, m : m + 1]
                )

            # staging for the whole flat output (lives on one partition)
            osb = cpool.tile([1, t_total * BS], F32)

            # chain PSUM accumulators, one per chain (persistent)
            zps = [
                pz_pool.tile([128, MB * BG], F32, name=f"zps{g}") for g in range(G)
            ]
            zpv = [
                z.rearrange("p (mb beta) -> p mb beta", mb=MB) for z in zps
            ]

            av, cv, uv = {}, {}, {}

            def phase_a_ops(b):
                """Closures computing A(b) and C(b); FIFO order = dep order."""
                t0 = b * TBLK
                ops = []
                # fp32 load on the HW DGE (the software DGE that can cast is
                # ~10x slower); GPSIMD then casts to bf16 for the transposes
                xbm32 = x32pool.tile([128, 4 * D], F32, name="xbm32")
                xbm32_v = xbm32.rearrange("p (i d) -> p i d", i=4)
                xbm = xpool.tile([128, 4 * D], BF, name="xbm")
                xbm_v = xbm.rearrange("p (i d) -> p i d", i=4)
                for i in range(4):
                    ops.append(lambda i=i: nc.sync.dma_start(
                        out=xbm32_v[:, i, :],
                        in_=x[:, t0 + 8 * i : t0 + 8 * (i + 1), :].rearrange(
                            "b tp d -> tp b d"
                        ),
                    ))
                    ops.append(lambda i=i: nc.gpsimd.tensor_copy(
                        xbm_v[:, i, :], xbm32_v[:, i, :]
                    ))
                xt = xtpool.tile([128, KB * 512], BF, name="xt")
                xt_v = xt.rearrange("p (kb bt) -> p kb bt", kb=KB)
                for i in range(4):
                    for dj in range(KB):
                        ops.append(lambda i=i, dj=dj: nc.sync.dma_start(
                            out=xt_v[:, dj, i * 128 : (i + 1) * 128],
                            in_=xbm_v[:, i, dj * 128 : (dj + 1) * 128],
                            transpose=True,
                        ))

                asb = apool.tile([128, TBLK * MB * BS], BF, name="asb")
                av[b] = asb.rearrange(
                    "p (tau mb beta) -> p tau mb beta", tau=TBLK, mb=MB
                )

                psa_t = {}

                def w1_mm(m, k):
                    if k == 0:
                        psa_t[m] = pa_pool.tile([128, 512], F32, name="psa")
                    nc.tensor.matmul(
                        psa_t[m], w1v[:, k, m, :], xt_v[:, k, :],
                        start=(k == 0), stop=(k == KB - 1),
                    )

                for m in range(MB):
                    for k in range(KB):
                        ops.append(lambda m=m, k=k: w1_mm(m, k))
                    ops.append(lambda m=m: nc.scalar.activation(
                        av[b][:, :, m, :],
                        psa_t[m].rearrange("p (tau beta) -> p tau beta", tau=TBLK),
                        TANH, bias=b1sb[:, m : m + 1],
                    ))

                csb = cblkpool.tile([128, TBLK * MB * BS], BF, name="csb")
                cv[b] = csb.rearrange(
                    "p (tau mb beta) -> p tau mb beta", tau=TBLK, mb=MB
                )

                psc_t = {}

                def c_mm(m, k):
                    if k == 0:
                        psc_t[m] = pc_pool.tile([128, 512], F32, name="psc")
                    nc.tensor.matmul(
                        psc_t[m], w2v[:, k, m, :], av[b][:, :, k, :],
                        start=(k == 0), stop=(k == KB - 1),
                    )

                for m in range(MB):
                    for k in range(KB):
                        ops.append(lambda m=m, k=k: c_mm(m, k))
                    ops.append(lambda m=m: nc.vector.tensor_scalar_add(
                        cv[b][:, :, m, :],
                        psc_t[m].rearrange("p (tau beta) -> p tau beta", tau=TBLK),
                        b2sb[:, m : m + 1],
                    ))
                return ops

            def phase_c_ops(b):
                """out_t = A_t.Wc + u_t.Wc for block b."""
                t0 = b * TBLK
                ops = []
                po_t = {}

                def o_mm(src, k):
                    if src == "u" and k == 0:
                        po_t[0] = po_pool.tile([1, TBLK * BS], F32, name="po")
                    v = uv[b] if src == "u" else av[b]
                    nc.tensor.matmul(
                        po_t[0], wcsb[:, k : k + 1], v[:, :, k, :],
                        start=(src == "u" and k == 0),
                        stop=(src == "a" and k == KB - 1),
                    )

                for k in range(KB):
                    ops.append(lambda k=k: o_mm("u", k))
                for k in range(KB):
                    ops.append(lambda k=k: o_mm("a", k))
                ops.append(lambda: nc.vector.tensor_copy(
                    osb[:, t0 * BS : (t0 + TBLK) * BS], po_t[0]
                ))
                return ops

            def chain_pre(b, tau, g):
                """Open zps[g]'s accumulation group with C_{t-1}."""
                gs = slice(g * BG, (g + 1) * BG)
                if b == 0 and tau == 0:
                    c_src_all = c0v[:, :, gs]
                elif tau == 0:
                    c_src_all = cv[b - 1][:, TBLK - 1, :, gs]
                else:
                    c_src_all = cv[b][:, tau - 1, :, gs]
                nc.tensor.matmul(
                    zpv[g], ident, c_src_all,
                    start=True, stop=False,
                )

            def chain_main(b, tau, g):
                gs = slice(g * BG, (g + 1) * BG)
                if b == 0 and tau == 0:
                    u_src = lambda k: u0v[:, k, gs]
                elif tau == 0:
                    u_src = lambda k: uv[b - 1][:, TBLK - 1, k, gs]
                else:
                    u_src = lambda k: uv[b][:, tau - 1, k, gs]
                for k in range(KB):
                    for m in range(MB):
                        nc.tensor.matmul(
                            zpv[g][:, m, :], w2v[:, k, m, :], u_src(k),
                            start=False, stop=(k == KB - 1 and m == MB - 1),
                        )
                nc.scalar.activation(uv[b][:, tau, :, gs], zpv[g], TANH)

            # block 0's phase A must fully precede its chains
            for op in phase_a_ops(0):
                op()

            pending = deque()
            for b in range(nblk):
                ublk = upool.tile([128, TBLK * MB * BS], BF, name="ublk")
                uv[b] = ublk.rearrange(
                    "p (tau mb beta) -> p tau mb beta", tau=TBLK, mb=MB
                )
                if b + 1 < nblk:
                    pending.extend(phase_a_ops(b + 1))
                if b >= 1:
                    pending.extend(phase_c_ops(b - 1))
                for tau in range(TBLK):
                    for g in range(G):
                        chain_pre(b, tau, g)
                        chain_main(b, tau, g)
                    take = -(-len(pending) // (TBLK - tau))
                    for _ in range(take):
                        pending.popleft()()
                # free stale views (pools recycle the buffers)
                for d in (av, cv, uv):
                    d.pop(b - 2, None)

            for op in phase_c_ops(nblk - 1):
                op()

            nc.sync.dma_start(out=out[:, :], in_=osb)

    nc.finalize()
    return nc


_CACHE = {}


def _get_nc(t_total):
    if t_total not in _CACHE:
        _CACHE[t_total] = build_nc(t_total)
    return _CACHE[t_total]


def _run(inputs, t_total, trace=False):
    nc = _get_nc(t_total)
    xf = np.ascontiguousarray(np.asarray(inputs["x"], dtype=np.float32))
    in_common = {
        k: np.ascontiguousarray(np.asarray(inputs[k], dtype=np.float32))
        for k in ("W1", "b1", "W2", "b2", "Wc")
    }
    in_maps = [
        {"x": xf[c * BS : (c + 1) * BS, :t_total], **in_common} for c in range(NCORES)
    ]
    res = run_bass_kernel_spmd(nc, in_maps, core_ids=list(range(NCORES)), trace=trace)
    outs = np.concatenate(
        [
            res.results[c]["out"].reshape(t_total, BS).T
            for c in range(NCORES)
        ],
        axis=0,
    )
    bc = np.asarray(inputs["bc"], dtype=np.float32)
    full = outs[:, :, None] + bc[None, None, :]
    return full.astype(np.float32), res


def kernel(**inputs):
    t_total = int(os.environ.get("RNN_T", str(T_FULL)))
    full, _ = _run(inputs, t_total, trace=False)
    return full
